# revision 9
# baseline (speedup 1.0000x reference)
"""Trainium2 Bass kernel for byte-to-patch cross attention.

Problem shapes (hardcoded): B=2, S=4096, P=1024, D=1024, H=16 heads, dh=64.

Sharding: 8 cores = batch (2) x head-groups (4). Core i handles batch i//4
and heads 4*(i%4) .. 4*(i%4)+3 (a 256-wide slice of the projection dims).
Each core computes q/k/v projections for its head slice, masked softmax
attention, and its partial output projection  O_g @ wo[:, g].T.  The host
sums the 4 partials per batch (the "all-reduce" of the tensor-parallel
output projection) and adds the bv/bo bias terms.

On-device layout notes:
 - All big operands are staged transposed by the host (contraction dim on
   partitions), so no on-device transposes are needed anywhere.
 - scores are computed transposed: S_h[p, t] = K_h @ Q_h^T, patches on
   partitions, bytes on the free axis.
 - softmax runs over the partition axis: exp on ACT, masking via one
   fp16 scalar_tensor_tensor per tile.
 - the softmax denominator rides along in the PV matmul: the stationary
   V tile per head is [V_h | ones] (M=65), so psum row 64 accumulates
   sum(probs) while rows 0..63 accumulate P@V.  The reciprocal of that
   row is broadcast across 64 partitions with a K=1 ones matmul, and the
   normalization happens during PSUM evacuation (one DVE multiply per
   head, the h1 copy writes partitions 64..127 from psum rows 0..63).
 - boundary (chunk, patch-tile) pairs restrict scores/exp/mask/PV
   to the visible byte suffix [tlo:]; fully-masked tiles are skipped.
 - Q-projection and the output projection are interleaved into the
   per-byte-chunk attention loop so the PE stays busy while ACT runs exp.
"""

import sys

sys.path.insert(0, "/opt/trn_rl_repo")

import numpy as np

import concourse.bass as bass
import concourse.mybir as mybir
from concourse import bacc, tile
from concourse.bass_utils import run_bass_kernel_spmd

B, S, P, D, H = 2, 4096, 1024, 1024, 16
HPC = H // 4          # heads per core = 4
GD = HPC * 64         # projection dim slice per core = 256
DH = 64               # head dim
SCALE = 1.0 / 8.0     # 1/sqrt(dh)

F16 = mybir.dt.float16
F32 = mybir.dt.float32

TC = 512              # byte-seq chunk (matmul free dim / psum bank)
NTC = S // TC         # 8
NPT = P // 128        # 8 patch tiles
NK = D // 128         # 8 contraction chunks for projections

_CACHE = {}


def _build_program(vis=None, tlo=None, repeat=1):
    nc = bacc.Bacc("TRN2", target_bir_lowering=False, debug=False)

    xt_d = nc.dram_tensor("xt", [D, S], F16, kind="ExternalInput")       # X^T
    rt_d = nc.dram_tensor("rt", [D, P], F16, kind="ExternalInput")       # R^T
    wqt_d = nc.dram_tensor("wqt", [D, GD], F16, kind="ExternalInput")    # wq_g^T
    wkt_d = nc.dram_tensor("wkt", [D, GD], F16, kind="ExternalInput")    # wk_g^T
    wvt_d = nc.dram_tensor("wvt", [D, GD], F16, kind="ExternalInput")    # wv_g^T
    wot_d = nc.dram_tensor("wot", [GD, D], F16, kind="ExternalInput")    # wo[:,g]^T
    cbc_d = nc.dram_tensor("cbc", [128, S], F16, kind="ExternalInput")   # cumsum bcast
    jcol_d = nc.dram_tensor("jcol", [128, NPT], F16, kind="ExternalInput")
    bqc_d = nc.dram_tensor("bqc", [128, 2], F32, kind="ExternalInput")
    bkc_d = nc.dram_tensor("bkc", [128, 2], F32, kind="ExternalInput")
    y_d = nc.dram_tensor("y", [S, D], F16, kind="ExternalOutput")

    with tile.TileContext(nc) as tc:
        with (
            tc.tile_pool(name="const", bufs=1) as cpool,
            tc.tile_pool(name="xt", bufs=1) as xt_pool,
            tc.tile_pool(name="rt", bufs=1) as rt_pool,
            tc.tile_pool(name="qt", bufs=2) as qt_pool,
            tc.tile_pool(name="kt", bufs=2) as kt_pool,
            tc.tile_pool(name="vp", bufs=NPT) as v_pool,
            tc.tile_pool(name="probs", bufs=28) as pr_pool,
            tc.tile_pool(name="ot", bufs=2) as ot_pool,
            tc.tile_pool(name="misc", bufs=4) as misc_pool,
            tc.tile_pool(name="yout", bufs=4) as y_pool,
            tc.tile_pool(name="ps_proj", bufs=2, space="PSUM") as ps_proj,
            tc.tile_pool(name="ps_sc", bufs=3, space="PSUM") as ps_sc,
            tc.tile_pool(name="ps_pv", bufs=1, space="PSUM") as ps_pv,
            tc.tile_pool(name="ps_bc", bufs=1, space="PSUM") as ps_bc,
        ):
          if vis is None:
            vis = [[1] * NPT for _ in range(NTC)]
          if tlo is None:
            tlo = [[0] * NPT for _ in range(NTC)]
          for _rep in range(repeat):
            # ---- loads, ordered so early compute phases unblock first ----
            # (tiny constants, then K-proj inputs, then Q-proj inputs)
            bkc = cpool.tile([128, 2], F32, tag="bkc")
            nc.sync.dma_start(bkc[:], bkc_d[:])
            bqc = cpool.tile([128, 2], F32, tag="bqc")
            nc.sync.dma_start(bqc[:], bqc_d[:])
            jcol = cpool.tile([128, NPT], F16, tag="jcol")
            nc.sync.dma_start(jcol[:], jcol_d[:])
            # per-k-chunk loads so the first projection matmuls can start
            # as soon as chunk 0 lands (subtile deps)
            wk_sb = cpool.tile([128, NK * GD], F16, tag="wk_sb")
            rt_sb = rt_pool.tile([128, NK * P], F16, name="rt_sb")
            for k in range(NK):
                nc.sync.dma_start(wk_sb[:, k * GD:(k + 1) * GD],
                                  wkt_d[k * 128:(k + 1) * 128, :])
                nc.sync.dma_start(rt_sb[:, k * P:(k + 1) * P],
                                  rt_d[k * 128:(k + 1) * 128, :])
            wq_sb = cpool.tile([128, NK * GD], F16, tag="wq_sb")
            for k in range(NK):
                nc.sync.dma_start(wq_sb[:, k * GD:(k + 1) * GD],
                                  wqt_d[k * 128:(k + 1) * 128, :])
            wv_sb = cpool.tile([128, NK * GD], F16, tag="wv_sb")
            for k in range(NK):
                nc.sync.dma_start(wv_sb[:, k * GD:(k + 1) * GD],
                                  wvt_d[k * 128:(k + 1) * 128, :])
            wqt = [wq_sb[:, k * GD:(k + 1) * GD] for k in range(NK)]
            wkt = [wk_sb[:, k * GD:(k + 1) * GD] for k in range(NK)]
            wvt = [wv_sb[:, k * GD:(k + 1) * GD] for k in range(NK)]
            rt = [rt_sb[:, k * P:(k + 1) * P] for k in range(NK)]

            # xt streamed per byte-chunk so Q-proj can start early
            xt_sb = xt_pool.tile([128, NK * S], F16, name="xt_sb")
            xt_v = xt_sb[:].rearrange("p (k c) -> p k c", k=NK)
            xt_dv = xt_d.rearrange("(k p) c -> p k c", p=128)

            def load_xt(tci):
                nc.sync.dma_start(xt_v[:, :, tci * TC:(tci + 1) * TC],
                                  xt_dv[:, :, tci * TC:(tci + 1) * TC])

            load_xt(0)
            load_xt(1)
            xt = [xt_sb[:, k * S:(k + 1) * S] for k in range(NK)]

            cbc = cpool.tile([128, S], F16, tag="cbc")
            nc.sync.dma_start(cbc[:], cbc_d[:])
            wo_sb = cpool.tile([128, 2 * D], F16, tag="wo_sb")
            nc.sync.dma_start(
                wo_sb[:].rearrange("p (k c) -> p k c", k=2),
                wot_d.rearrange("(k p) c -> p k c", p=128))
            wot = [wo_sb[:, k * D:(k + 1) * D] for k in range(2)]
            for tci in range(2, NTC):
                load_xt(tci)
            ones64 = cpool.tile([128, 64], F16, tag="ones64")
            nc.vector.memset(ones64[:], 1.0)

            # ---- K^T, V projections (from R^T) ---------------------------
            # K^T [GD, P] as 2 sbuf tiles; scale 1/8 and bias folded in.
            kt = [kt_pool.tile([128, P], F16, name="kt_t") for _ in range(2)]
            for m in range(2):
                for pc in range(P // TC):
                    pk = ps_proj.tile([128, TC], F32, tag="pj")
                    for k in range(NK):
                        nc.tensor.matmul(
                            pk[:],
                            wkt[k][:, m * 128:(m + 1) * 128],
                            rt[k][:, pc * TC:(pc + 1) * TC],
                            start=(k == 0),
                            stop=(k == NK - 1),
                        )
                    nc.vector.tensor_scalar(
                        kt[m][:, pc * TC:(pc + 1) * TC], pk[:],
                        bkc[:, m:m + 1], SCALE,
                        op0=mybir.AluOpType.add, op1=mybir.AluOpType.mult,
                    )

            # V [P, GD+4] natural, 8 tiles of [128, 260]: per head lh the
            # stationary PV operand is cols [65*lh, 65*lh+65) = [V_lh | ones];
            # column 65*lh+64 stays 1.0 from the memset so psum row 64
            # accumulates the softmax denominator.
            vt = [v_pool.tile([128, 65 * HPC], F16, name="v_t")
                  for _ in range(NPT)]
            for pt in range(NPT):
                nc.vector.memset(vt[pt][:], 1.0)
            for pt in range(NPT):
                pv = ps_proj.tile([128, GD], F32, tag="pj")
                for k in range(NK):
                    nc.tensor.matmul(
                        pv[:],
                        rt[k][:, pt * 128:(pt + 1) * 128],
                        wvt[k],
                        start=(k == 0),
                        stop=(k == NK - 1),
                    )
                nc.vector.tensor_copy(
                    vt[pt][:].rearrange("p (l c) -> p l c", l=HPC)[:, :, 0:64],
                    pv[:].rearrange("p (l c) -> p l c", l=HPC),
                )

            # ---- Q^T projection (helper, emitted per byte-chunk) ---------
            qt = [qt_pool.tile([128, S], F16, name="qt_t") for _ in range(2)]

            def qproj(tc_i):
                for m in range(2):
                    pq = ps_proj.tile([128, TC], F32, tag="pj")
                    for k in range(NK):
                        nc.tensor.matmul(
                            pq[:],
                            wqt[k][:, m * 128:(m + 1) * 128],
                            xt[k][:, tc_i * TC:(tc_i + 1) * TC],
                            start=(k == 0),
                            stop=(k == NK - 1),
                        )
                    nc.vector.tensor_scalar_add(
                        qt[m][:, tc_i * TC:(tc_i + 1) * TC], pq[:], bqc[:, m:m + 1]
                    )

            ot_tiles = [ot_pool.tile([128, S], F16, name="ot_t") for _ in range(2)]

            def attn_group(tc_i, g2):
                # scores + probs for head pair g2 (local heads 2g2, 2g2+1)
                tsl = slice(tc_i * TC, (tc_i + 1) * TC)
                probs = [[None] * NPT for _ in range(2)]
                live_pt = [pt for pt in range(NPT) if vis[tc_i][pt] > 0]
                rlo = [0] * NPT
                for pt in live_pt:
                    # bytes < r0 in this chunk cannot see patch tile pt
                    # in any batch: skip them in scores/exp/mask/PV.
                    r0 = tlo[tc_i][pt] if vis[tc_i][pt] == 1 else 0
                    rlo[pt] = r0
                    rsl = slice(tc_i * TC + r0, (tc_i + 1) * TC)
                    for hh in range(2):  # head within pair
                        base = hh * 64
                        psc = ps_sc.tile([128, TC], F32, tag="sc")
                        nc.tensor.matmul(
                            psc[:, r0:],
                            kt[g2][base:base + 64, pt * 128:(pt + 1) * 128],
                            qt[g2][base:base + 64, rsl],
                            start=True, stop=True,
                            tile_position=(base, 0),
                        )
                        pr = pr_pool.tile([128, TC], F16, tag="pr")
                        # exp then mask: probs = (cbc >= j) * exp(scores)
                        nc.scalar.activation(
                            pr[:, r0:], psc[:, r0:],
                            mybir.ActivationFunctionType.Exp
                        )
                        if vis[tc_i][pt] == 1:
                            nc.vector.scalar_tensor_tensor(
                                pr[:, r0:], cbc[:, rsl], jcol[:, pt:pt + 1],
                                pr[:, r0:],
                                op0=mybir.AluOpType.is_ge,
                                op1=mybir.AluOpType.mult,
                            )
                        probs[hh][pt] = pr

                # PV matmuls with the denominator riding in psum row 64
                ppv = ps_pv.tile([65, 2, TC], F32, tag="pv", name="ppv")
                for pt in live_pt:
                    r0 = rlo[pt]
                    for hh in range(2):
                        lh = 2 * g2 + hh
                        nc.tensor.matmul(
                            ppv[:, hh, r0:],
                            vt[pt][:, 65 * lh:65 * lh + 65],
                            probs[hh][pt][:, r0:],
                            start=(pt == live_pt[0]), stop=(pt == live_pt[-1]),
                            skip_group_check=True,
                        )
                # reciprocal of the denominator rows -> partition 0
                recip = misc_pool.tile([1, 2 * TC], F16, tag="recip")
                with nc.allow_low_precision(reason="softmax recip, fp16 ok"):
                    nc.vector.reciprocal(recip[0:1, :], ppv[64:65, :, :])
                # broadcast 1/den across 64 partitions (K=1 ones matmul),
                # then normalize during psum evacuation; h1 writes parts 64+
                bc_sb = misc_pool.tile([64, 2 * TC], F16, tag="bc_sb")
                for hh in range(2):
                    pbc = ps_bc.tile([64, TC], F32, tag="bc", name="pbc")
                    nc.tensor.matmul(
                        pbc[:, :], ones64[0:1, :],
                        recip[0:1, hh * TC:(hh + 1) * TC],
                        start=True, stop=True,
                    )
                    nc.vector.tensor_copy(
                        bc_sb[:, hh * TC:(hh + 1) * TC], pbc[:, :])
                nc.vector.tensor_mul(
                    ot_tiles[g2][0:64, tsl], ppv[0:64, 0, :], bc_sb[:, 0:TC])
                nc.vector.tensor_mul(
                    ot_tiles[g2][64:128, tsl], ppv[0:64, 1, :], bc_sb[:, TC:])

            def oproj(tc_i):
                # output projection for byte chunk tc_i (4 t-tiles of 128);
                # psum evacuation alternates DVE / ACT to balance engines
                for tt in range(4):
                    t0 = tc_i * TC + tt * 128
                    ysb = y_pool.tile([128, D], F16, tag="y")
                    for n in range(2):
                        py = ps_proj.tile([128, TC], F32, tag="pj")
                        for k2 in range(2):
                            nc.tensor.matmul(
                                py[:],
                                ot_tiles[k2][:, t0:t0 + 128],
                                wot[k2][:, n * TC:(n + 1) * TC],
                                start=(k2 == 0), stop=(k2 == 1),
                            )
                        if n == 1 and tt % 2 == 1:
                            nc.scalar.copy(ysb[:, n * TC:(n + 1) * TC], py[:])
                        else:
                            nc.vector.tensor_copy(
                                ysb[:, n * TC:(n + 1) * TC], py[:])
                    nc.sync.dma_start(y_d[t0:t0 + 128, :], ysb[:])

            # ---- main loop: attention with q/o projections interleaved ---
            qproj(0)
            qproj(1)
            for tc_i in range(NTC):
                attn_group(tc_i, 0)
                if tc_i > 0:
                    oproj(tc_i - 1)
                attn_group(tc_i, 1)
                if tc_i + 2 < NTC:
                    qproj(tc_i + 2)
            oproj(NTC - 1)

    nc.compile()
    return nc


def _vis_plan(patch_boundaries):
    """vis[tc][pt]: 0 = fully masked in every batch (skip), 2 = fully
    visible in every batch (no mask op), 1 = boundary (apply mask).
    tlo[tc][pt]: first byte offset within the chunk at which patch tile
    pt becomes visible in ANY batch (bytes before it are fully masked).
    Must be valid for all cores, i.e. union over batches."""
    cs = np.cumsum(patch_boundaries, axis=1)  # [B, S]
    vis, tlo = [], []
    for tci in range(NTC):
        lo = cs[:, tci * TC].min()
        hi = cs[:, (tci + 1) * TC - 1].max()
        row, trow = [], []
        for pt in range(NPT):
            if pt * 128 > hi:
                row.append(0)
                trow.append(0)
            elif (pt + 1) * 128 - 1 <= lo:
                row.append(2)
                trow.append(0)
            else:
                row.append(1)
                # first global byte index where any batch reaches pt*128
                t_cross = min(int(np.searchsorted(cs[b], pt * 128, "left"))
                              for b in range(cs.shape[0]))
                trow.append(max(0, min(t_cross - tci * TC, TC - 1)))
        vis.append(row)
        tlo.append(trow)
    return vis, tlo


def _get_program(plan=None, repeat=1):
    vis, tlo = plan if plan else (None, None)
    key = (tuple(tuple(r) for r in vis) if vis else None,
           tuple(tuple(r) for r in tlo) if tlo else None, repeat)
    if key not in _CACHE:
        _CACHE[key] = _build_program(vis, tlo, repeat)
    return _CACHE[key]


def _prep_inputs(queries, patch_representations, patch_boundaries,
                 wq, wk, wv, wo, bq, bk):
    """Build the 8 per-core input maps."""
    in_maps = []
    jcol = (np.arange(128, dtype=np.float32)[:, None]
            + 128.0 * np.arange(NPT, dtype=np.float32)[None, :]).astype(np.float16)
    for core in range(8):
        b, g = core // 4, core % 4
        sl = slice(g * GD, (g + 1) * GD)
        c = np.cumsum(patch_boundaries[b]).astype(np.float32)
        cbc = np.broadcast_to(c.astype(np.float16), (128, S)).copy()
        in_maps.append({
            "xt": np.ascontiguousarray(queries[b].T).astype(np.float16),
            "rt": np.ascontiguousarray(patch_representations[b].T).astype(np.float16),
            "wqt": np.ascontiguousarray(wq[sl, :].T).astype(np.float16),
            "wkt": np.ascontiguousarray(wk[sl, :].T).astype(np.float16),
            "wvt": np.ascontiguousarray(wv[sl, :].T).astype(np.float16),
            "wot": np.ascontiguousarray(wo[:, sl].T).astype(np.float16),
            "cbc": cbc,
            "jcol": jcol,
            "bqc": np.ascontiguousarray(bq[sl].reshape(2, 128).T).astype(np.float32),
            "bkc": np.ascontiguousarray(bk[sl].reshape(2, 128).T).astype(np.float32),
        })
    return in_maps


def _reduce_outputs(results, wo, bv, bo):
    y = np.zeros((B, S, D), dtype=np.float32)
    for core in range(8):
        y[core // 4] += results[core]["y"].astype(np.float32)
    y += (bv @ wo.T + bo)[None, None, :]
    return y


def kernel(queries, patch_representations, patch_boundaries,
           wq, wk, wv, wo, bq, bk, bv, bo):
    queries = np.asarray(queries, dtype=np.float32)
    patch_representations = np.asarray(patch_representations, dtype=np.float32)
    patch_boundaries = np.asarray(patch_boundaries)
    wq, wk, wv, wo = (np.asarray(a, dtype=np.float32) for a in (wq, wk, wv, wo))
    bq, bk, bv, bo = (np.asarray(a, dtype=np.float32) for a in (bq, bk, bv, bo))
    nc = _get_program(_vis_plan(patch_boundaries))
    in_maps = _prep_inputs(queries, patch_representations, patch_boundaries,
                           wq, wk, wv, wo, bq, bk)
    res = run_bass_kernel_spmd(nc, in_maps, core_ids=list(range(8)))
    return _reduce_outputs(res.results, wo, bv, bo)
